# revision 16
# baseline (speedup 1.0000x reference)
"""Causal self-attention (B=2, T=2048, E=1024, H=16, D=64) on 8 NeuronCores.

Sharding: batch (2) x head-groups (4 groups of 4 heads) -> 8 cores.
Each core computes, for its (batch b, head group g):
  Q^T/K^T = (Wq/Wk col-shard)^T @ x_b^T        (heads on partitions, t free)
  V       = x_b @ Wv col-shard                 (tokens on partitions)
  S^T     = K^T-slices^T-matmuls (s on partitions, t free), causal
  P^T     = exp(S^T/8) (no max subtraction: scores ~ N(0,1), exp is safe)
  U^T,r   = [V | ones]^T @ P^T  (PV product + row-sums in one matmul)
  A^T     = U^T * (1/r)                        (softmax normalization)
  Ypart   = A^T-slices^T @ Wo row-shard        (partial out-proj)
Host sums the 4 partials per batch and adds the constant row
bo + bv @ Wo (exact: v-bias passes through attention unchanged; the k-bias
is softmax-invariant and dropped; q-bias is applied to Q on device).

v2 performance structure:
- Attention-path operands (Q^T/K^T/A^T/V/P^T and Wo) are bf16: same PE
  stream rate as f32r for wide matmuls, but no 4x fp32r penalty on the
  narrow (<256 free dim) diagonal-block matmuls, and half the SBUF/DMA
  traffic. Projections stay f32r (full accuracy, already 1 cycle/row).
- Software-pipelined emission: the per-engine queues run in program
  order, so chunk c's attention j-loop (Act-engine exp-bound) has chunk
  c+1's projection matmul chains interleaved into it to keep the PE busy
  and at full p-state.
- Engine balance: Act = exp + y PSUM->SBUF copies; DVE = Q bias-add,
  K copy, softmax reciprocal, A normalization mul; Pool = V copies,
  causal masks, 1/r partition broadcasts; PE = matmuls only.
- Per-chunk K/V/Q/A tiles (not one big tensor) so cross-chunk writes
  never alias reads and the tile dependency tracker can't serialize the
  pipeline.
"""
import sys

if "/opt/trn_rl_repo" not in sys.path:
    sys.path.insert(0, "/opt/trn_rl_repo")

import numpy as np

import concourse.bass as bass
import concourse.mybir as mybir
import concourse.tile as tile
from concourse import bacc
from concourse.bass_utils import run_bass_kernel_spmd

F32 = mybir.dt.float32
F32R = mybir.dt.float32r
BF16 = mybir.dt.bfloat16

B, T, E = 2, 2048, 1024
H, D = 16, 64
N_CORES = 8
HEADS_PER_CORE = 4            # 16 heads / 4 groups
EPC = HEADS_PER_CORE * D      # 256: e' columns per core
TC = 512                      # t-chunk (psum free width)
NTC = T // TC                 # 4 t-chunks
SB = 128                      # s-block (psum partitions)
KCH = E // 128                # 8 contraction chunks


def build_kernel(repeat: int = 1) -> bass.Bass:
    """repeat>1 wraps the whole compute in a hardware loop — used only for
    wall-clock timing (the axon round-trip is ~91ms, so per-iteration time
    is measured as (wall(R) - wall(1)) / (R - 1))."""
    nc = bacc.Bacc(None, target_bir_lowering=False, debug=False)

    xT = nc.dram_tensor("xT", [E, T], F32, kind="ExternalInput")
    wq = nc.dram_tensor("wq", [E, EPC], F32, kind="ExternalInput")
    wk = nc.dram_tensor("wk", [E, EPC], F32, kind="ExternalInput")
    wv = nc.dram_tensor("wv", [E, EPC], F32, kind="ExternalInput")
    wo = nc.dram_tensor("wo", [EPC, E], F32, kind="ExternalInput")
    bq = nc.dram_tensor("bq", [EPC], F32, kind="ExternalInput")
    y = nc.dram_tensor("y", [T, E], F32, kind="ExternalOutput")

    with tile.TileContext(nc) as tc:
        with tc.tile_pool(name="singles", bufs=1) as singles, \
             tc.tile_pool(name="xtp", bufs=3) as xtp, \
             tc.tile_pool(name="qtp", bufs=3) as qtp, \
             tc.tile_pool(name="atp", bufs=2) as atp, \
             tc.tile_pool(name="pp", bufs=6) as pp, \
             tc.tile_pool(name="rp", bufs=4) as rp, \
             tc.tile_pool(name="rbp", bufs=4) as rbp, \
             tc.tile_pool(name="ysb", bufs=4) as ysbp, \
             tc.tile_pool(name="ps_a", bufs=2, space="PSUM") as ps_a, \
             tc.tile_pool(name="ps_s", bufs=2, space="PSUM") as ps_s, \
             tc.tile_pool(name="ps_u", bufs=2, space="PSUM") as ps_u:

            # ---- weight / bias loads (once) ----
            wq_sb = singles.tile([128, KCH, EPC], F32R, tag="wq")
            wk_sb = singles.tile([128, KCH, EPC], F32R, tag="wk")
            wv_sb = singles.tile([128, KCH, EPC], F32R, tag="wv")
            for k in range(KCH):
                nc.sync.dma_start(out=wq_sb[:, k, :],
                                  in_=wq[k * 128:(k + 1) * 128, :].bitcast(F32R))
                nc.sync.dma_start(out=wk_sb[:, k, :],
                                  in_=wk[k * 128:(k + 1) * 128, :].bitcast(F32R))
                nc.sync.dma_start(out=wv_sb[:, k, :],
                                  in_=wv[k * 128:(k + 1) * 128, :].bitcast(F32R))
            # wo: head h lives at partitions 64*(h%2).. of slab h//2.
            # Loaded as f32 and cast on device (bf16 external inputs don't
            # survive the pjrt input path).
            wo_f = singles.tile([128, 2, E], F32, tag="wof")
            wo_sb = singles.tile([128, 2, E], BF16, tag="wo")
            for j in range(2):
                nc.sync.dma_start(out=wo_f[:, j, :],
                                  in_=wo[j * 128:(j + 1) * 128, :])
            nc.scalar.copy(wo_sb[:], wo_f[:])
            bq_sb = singles.tile([128, 2], F32, tag="bq")
            for eh in range(2):
                nc.sync.dma_start(out=bq_sb[:, eh],
                                  in_=bq[eh * 128:(eh + 1) * 128])

            # ---- persistent per-chunk activations ----
            # KT[c]: [d-pair partitions, eh slab, t in chunk]; heads (2eh,
            # 2eh+1) on partition halves.  V[c][s_part, s_blk, head,
            # 0:64]=V, [...,64]=1.0 (PV row-sum trick).
            KTs = [singles.tile([128, 2, TC], BF16, tag=f"KT{c}",
                                name=f"KT{c}") for c in range(NTC)]
            Vs = [singles.tile([128, TC // SB, HEADS_PER_CORE, D + 1], BF16,
                               tag=f"V{c}", name=f"V{c}") for c in range(NTC)]
            for c in range(NTC):
                nc.gpsimd.memset(Vs[c][:, :, :, D:D + 1], 1.0)

            # causal mask for diagonal s-blocks (same triangle for every
            # diagonal block: keep column x >= partition s). Built in f32 —
            # affine_select's iota is computed in the tensor dtype on
            # hardware, and bf16 can't represent integers > 256 — then cast.
            tri_f = singles.tile([128, 2, TC], F32, tag="trif")
            tri = singles.tile([128, 2, TC], BF16, tag="tri")
            nc.gpsimd.memset(tri_f[:], 1.0)
            nc.gpsimd.affine_select(
                out=tri_f[:], in_=tri_f[:],
                compare_op=mybir.AluOpType.is_ge, fill=0.0,
                base=0, pattern=[[0, 2], [1, TC]], channel_multiplier=-1)
            nc.vector.tensor_copy(tri[:], tri_f[:])

            xts = [None] * NTC
            qts = [None] * NTC

            def dma_xt(c):
                xt = xtp.tile([128, KCH, TC], F32R, tag="xt", name=f"xt{c}")
                t0 = c * TC
                for k in range(KCH):
                    nc.sync.dma_start(
                        out=xt[:, k, :],
                        in_=xT[k * 128:(k + 1) * 128, t0:t0 + TC].bitcast(F32R))
                xts[c] = xt

            def chain_q(c, eh):
                if eh == 0:
                    qts[c] = qtp.tile([128, 2, TC], BF16, tag="qt",
                                      name=f"qt{c}")
                ps = ps_a.tile([128, TC], F32, tag="a", name=f"q{c}_{eh}")
                for k in range(KCH):
                    nc.tensor.matmul(
                        ps[:], wq_sb[:, k, eh * 128:(eh + 1) * 128],
                        xts[c][:, k, :], start=(k == 0), stop=(k == KCH - 1))
                nc.vector.tensor_scalar_add(
                    out=qts[c][:, eh, :], in0=ps[:],
                    scalar1=bq_sb[:, eh:eh + 1])

            def chain_k(c, eh):
                ps = ps_a.tile([128, TC], F32, tag="a", name=f"k{c}_{eh}")
                for k in range(KCH):
                    nc.tensor.matmul(
                        ps[:], wk_sb[:, k, eh * 128:(eh + 1) * 128],
                        xts[c][:, k, :], start=(k == 0), stop=(k == KCH - 1))
                nc.vector.tensor_copy(KTs[c][:, eh, :], ps[:])

            def chain_v(c, j4):
                ps = ps_a.tile([128, EPC], F32, tag="a", name=f"v{c}_{j4}")
                for k in range(KCH):
                    nc.tensor.matmul(
                        ps[:], xts[c][:, k, j4 * SB:(j4 + 1) * SB],
                        wv_sb[:, k, :], start=(k == 0), stop=(k == KCH - 1))
                # gpsimd can't read PSUM; act has slack outside the exps
                nc.scalar.copy(
                    Vs[c][:, j4, :, 0:D],
                    ps[:].rearrange("p (h d) -> p h d", h=HEADS_PER_CORE))

            def chains(c):
                out = []
                for eh in range(2):
                    out.append(lambda eh=eh: chain_q(c, eh))
                    out.append(lambda eh=eh: chain_k(c, eh))
                for j4 in range(TC // SB):
                    out.append(lambda j4=j4: chain_v(c, j4))
                return out

            ats = [None] * NTC

            def c_unit(c, tb4, e):
                # one out-proj tile for chunk c: y[t-block, e-half]
                t0 = c * TC
                tb0 = t0 + tb4 * SB
                y_ps = ps_a.tile([128, 512], F32, tag="a")
                # contraction over e' = pair-slab partitions: one K=128
                # matmul per slab (2 heads at once)
                for eh in range(2):
                    nc.tensor.matmul(
                        y_ps[:],
                        ats[c][:, eh, tb4 * SB:(tb4 + 1) * SB],
                        wo_sb[:, eh, e * 512:(e + 1) * 512],
                        start=(eh == 0), stop=(eh == 1))
                y_sb = ysbp.tile([128, 512], F32, tag="ysb")
                nc.vector.tensor_copy(y_sb[:], y_ps[:])
                nc.sync.dma_start(
                    out=y[tb0:tb0 + SB, e * 512:(e + 1) * 512], in_=y_sb[:])

            def c_units(c):
                out = []
                for tb4 in range(TC // SB):
                    for e in range(2):
                        out.append(
                            lambda tb4=tb4, e=e: c_unit(c, tb4, e))
                return out

            def emit_body():
                # prologue: projections for chunk 0 un-pipelined
                dma_xt(0)
                for ch in chains(0):
                    ch()
                dma_xt(1)

                for c in range(NTC):
                    t0 = c * TC
                    nblk = (c + 1) * (TC // SB)
                    # PE filler for the act-paced j-loop: chunk c+1's
                    # projection chains (2 reserved for the tail window) and
                    # chunk c-1's deferred out-proj units, interleaved
                    pend = chains(c + 1) if c + 1 < NTC else []
                    cu = c_units(c - 1) if c > 0 else []
                    ch_in = pend[:6]
                    inline = []
                    for i in range(max(len(ch_in), len(cu))):
                        if i < len(cu):
                            inline.append(cu[i])
                        if i < len(ch_in):
                            inline.append(ch_in[i])
                    tail = pend[6:]
                    nj = 2 * nblk
                    pts = [int(round((i + 1) * nj / (len(inline) + 1)))
                           for i in range(len(inline))]
                    jc = 0

                    for eh in range(2):
                        u2 = [ps_u.tile([D + 1, TC], F32, tag="u",
                                        name=f"u{c}_{eh}_{h2}")
                              for h2 in range(2)]
                        for j in range(nblk):
                            # valid columns of this t-chunk: t >= j*SB
                            off = max(0, j * SB - t0)
                            w = TC - off
                            cj, bj = divmod(j, TC // SB)
                            s2 = ps_s.tile([128, 2, TC], F32, tag="s2")
                            for h2 in range(2):
                                r0 = 64 * h2
                                nc.tensor.matmul(
                                    s2[:, h2, off:],
                                    KTs[cj][r0:r0 + 64, eh,
                                            bj * SB:(bj + 1) * SB],
                                    qts[c][r0:r0 + 64, eh, off:],
                                    start=True, stop=True)
                            p2 = pp.tile([128, 2, TC], BF16, tag="pj")
                            nc.scalar.activation(
                                p2[:, :, off:], s2[:, :, off:],
                                mybir.ActivationFunctionType.Exp, scale=0.125)
                            if j >= c * (TC // SB):
                                # causal triangle: zero p where s > t.
                                # On DVE (2x bf16 mode), not Pool — Pool
                                # tensor ops lower to slow gpsimd ucode on
                                # real HW even though CoreSim models them
                                # at 1.2GHz.
                                nc.vector.tensor_mul(
                                    p2[:, :, off:], p2[:, :, off:],
                                    tri[:, :, 0:w])
                            for h2 in range(2):
                                nc.tensor.matmul(
                                    u2[h2][:, off:],
                                    Vs[cj][:, bj, 2 * eh + h2, :],
                                    p2[:, h2, off:],
                                    start=(j == 0), stop=(j == nblk - 1))
                            jc += 1
                            while pts and jc >= pts[0] and inline:
                                pts.pop(0)
                                inline.pop(0)()

                        # softmax normalization: A^T = U^T * (1/r)
                        if eh == 0:
                            ats[c] = atp.tile([128, 2, TC], BF16, tag="at",
                                              name=f"at{c}")
                        for h2 in range(2):
                            r0 = 64 * h2
                            # custom-DVE ucode can't read PSUM on HW: stage
                            # the row-sum into SBUF before the reciprocal
                            r_row = rp.tile([1, TC], F32, tag="rr")
                            nc.vector.tensor_copy(
                                r_row[:], u2[h2][D:D + 1, :])
                            rinv = rp.tile([1, TC], F32, tag="ri")
                            nc.vector.reciprocal_approx_fast(
                                out=rinv[:], in_=r_row[:])
                            rb = rbp.tile([64, TC], F32, tag="rb")
                            nc.gpsimd.partition_broadcast(rb[:], rinv[:])
                            nc.vector.tensor_mul(
                                ats[c][r0:r0 + 64, eh, :],
                                u2[h2][0:D, :], rb[:])

                    for ch in inline:
                        ch()
                    for ch in tail:
                        ch()
                    if c + 2 < NTC:
                        dma_xt(c + 2)

                # last chunk's out-proj has no later j-loop to hide in
                for ch in c_units(NTC - 1):
                    ch()

            if repeat == 1:
                emit_body()
            else:
                with tc.For_i(0, repeat, 1):
                    emit_body()

    nc.compile()
    return nc


_NC_CACHE = {}


def _get_nc(repeat: int = 1):
    if repeat not in _NC_CACHE:
        _NC_CACHE[repeat] = build_kernel(repeat)
    return _NC_CACHE[repeat]


def make_in_maps(inputs: dict) -> list:
    x = np.asarray(inputs["x"], dtype=np.float32)
    Wq = np.asarray(inputs["Wq"], dtype=np.float32)
    Wk = np.asarray(inputs["Wk"], dtype=np.float32)
    Wv = np.asarray(inputs["Wv"], dtype=np.float32)
    Wo = np.asarray(inputs["Wo"], dtype=np.float32)
    bq = np.asarray(inputs["bq"], dtype=np.float32)

    in_maps = []
    for core in range(N_CORES):
        b, g = divmod(core, N_CORES // B)
        cs = slice(g * EPC, (g + 1) * EPC)
        in_maps.append({
            "xT": np.ascontiguousarray(x[b].T),
            "wq": np.ascontiguousarray(Wq[:, cs]),
            "wk": np.ascontiguousarray(Wk[:, cs]),
            "wv": np.ascontiguousarray(Wv[:, cs]),
            "wo": np.ascontiguousarray(Wo[cs, :]),
            "bq": np.ascontiguousarray(bq[cs]),
        })
    return in_maps


def run_sharded(inputs: dict, trace: bool = False):
    """Shard inputs, run the SPMD kernel on 8 cores, unshard. Returns
    (output (B,T,E) float32, BassKernelResults)."""
    Wo = np.asarray(inputs["Wo"], dtype=np.float32)
    bv = np.asarray(inputs["bv"], dtype=np.float32)
    bo = np.asarray(inputs["bo"], dtype=np.float32)

    in_maps = make_in_maps(inputs)
    res = run_bass_kernel_spmd(_get_nc(), in_maps, core_ids=list(range(N_CORES)),
                               trace=trace)

    # unshard: sum the 4 head-group partials per batch; add the constant row
    # bo + bv @ Wo (v-bias commutes through the attention average exactly).
    const_row = (bo.astype(np.float64)
                 + bv.astype(np.float64) @ Wo.astype(np.float64))
    out = np.empty((B, T, E), dtype=np.float32)
    for b in range(B):
        acc = np.zeros((T, E), dtype=np.float64)
        for g in range(N_CORES // B):
            acc += res.results[b * (N_CORES // B) + g]["y"].astype(np.float64)
        out[b] = (acc + const_row).astype(np.float32)
    return out, res


def kernel(**inputs) -> np.ndarray:
    out, _ = run_sharded(inputs, trace=False)
    return out
